# revision 33
# baseline (speedup 1.0000x reference)
"""LoRA QKV fused projection kernel for 8 TRN2 NeuronCores.

Reference computation (T=8192 tokens, HID=4096, D=6144 out, S=8 slots, R=16):
    y = x @ W.T
    a[t,s,i,r] = sum_h x[t,h] * lora_A[s,i,r,h]          (down-proj, all slots)
    a *= onehot(token_to_slot)[t,s] * scaling[s]          (routing gate)
    d[t, :] = concat_i( sum_{s,r} a[t,s,i,r] * B_i[s,:,r] )   (up-proj)
    out = y + d

Sharding: pure 8-way token split (core c owns tokens [c*1024,(c+1)*1024),
full output dim).  This makes the per-core PE work exactly total/8 — no
LoRA down-proj duplication — at the cost of streaming the full W
(50 MB bf16) per core, which hides under the ~725 us of matmul.

All matmul operands are bf16 with fp32 PSUM accumulate (the TRN2 PE runs
1 column/cycle at 128x128 for bf16 AND fp32r, so bf16 buys DMA/SBUF
headroom, not raw PE rate; rel err ~3e-3 vs the 2e-2 gate).  Measured
~748 us/core on HW, >96% TensorMatrix occupancy:
  * ~110 tiny warm-up matmuls lift the HAM clock gate to 8/8 (2.4 GHz)
    while the first input chunks stream in.
  * x and lora_A stream on the sync queue in geometric k-chunks
    (2,2,4,4,4,8,8 k-tiles) behind which phase A starts computing.
  * Phase A: aT[i] = A_i @ x^T accumulated over 32 k-tiles into 3
    two-bank PSUM tiles, with row-block 0's W k-loop fused in (8 MMs
    per k-tile keeps PE consumption behind the DMA stream -> no
    stalls); then gated (onehot*scaling, host-built fp32) into bf16
    ag tiles.
  * Main loop over row-blocks 1..47: y-tile accumulated over 32
    k-tiles with W stationary (W prefetched 3-4 deep on sync), then
    ONE extra matmul accumulates the LoRA delta (B stationary, ag
    moving) into the same PSUM tile, one 2-bank fp32->bf16 cast
    (Vector/Scalar alternating) and one store per row-block.
  * Manually rotated tile sets (4 PSUM / 4 W / 3 st) bound the
    end-of-context semaphore drain.
Host: pure layout rearranges; final assembly is one cast+transpose per
block (no reduce, no permutation).
"""

import numpy as np
import ml_dtypes

# problem shape (hardcoded per harness contract)
T = 8192
HID = 4096
Q_SIZE = 4096
KV_SIZE = 1024
D = Q_SIZE + 2 * KV_SIZE  # 6144
S = 8
R = 16
NCORES = 8
P = 128

TC = T // NCORES          # 1024 tokens per core
MB = D // P               # 48 output row-blocks of 128
KA = HID // P             # 32 k-tiles
NJ = TC // 512            # 2 moving n-tiles of 512 tokens
KCH = 8                   # k-tiles per streamed input chunk
I_OF_MB = [0] * (Q_SIZE // P) + [1] * (KV_SIZE // P) + [2] * (KV_SIZE // P)

BF16 = np.dtype(ml_dtypes.bfloat16)

_CACHE = {}


def _build_nc():
    import concourse.mybir as mybir
    import concourse.tile as tile
    from concourse import bacc

    bf16 = mybir.dt.bfloat16
    f32 = mybir.dt.float32

    nc = bacc.Bacc(None, target_bir_lowering=False, debug=False)

    x_d = nc.declare_dram_parameter("x_sh", [P, KA, TC], bf16, isOutput=False)
    w_d = nc.declare_dram_parameter("w_sh", [MB, P, KA, P], bf16, isOutput=False)
    a_d = nc.declare_dram_parameter("a_sh", [P, KA, 3, P], bf16, isOutput=False)
    b_d = nc.declare_dram_parameter("b_sh", [P, MB, P], bf16, isOutput=False)
    g_d = nc.declare_dram_parameter("gate", [P, TC], f32, isOutput=False)
    y_d = nc.declare_dram_parameter("y_out", [MB, P, TC], bf16, isOutput=True)

    with tile.TileContext(nc) as tc:
        with tc.tile_pool(name="xres", bufs=1) as xres_pool, \
             tc.tile_pool(name="wp", bufs=1) as w_pool, \
             tc.tile_pool(name="cst", bufs=1) as cst_pool, \
             tc.tile_pool(name="agp", bufs=1) as ag_pool, \
             tc.tile_pool(name="stp", bufs=1) as st_pool, \
             tc.tile_pool(name="psum", bufs=1, space="PSUM") as ps_pool:

            # PE warm-up during the initial DMA bubble: ~3us of tiny matmuls
            # lifts the HAM clock gate to 8/8 before the first real matmul
            # (otherwise the first ~16 real MMs run at 1.2 GHz).  Emitted
            # before the DMA issues so the tensor queue starts immediately.
            # PSUM tiles are 2 banks (1024 fp32): one cast + one store per
            # row-block halves the end-of-kernel semaphore drain.
            # manually rotated tile sets (instead of per-iteration pool
            # allocations) keep the end-of-context semaphore drain short
            pbufs = [
                ps_pool.tile([P, 2 * 512], f32, tag=f"ps{q}", name=f"ps{q}")
                for q in range(4)
            ]
            wbufs = [
                w_pool.tile([P, KA, P], bf16, tag=f"w{q}", name=f"w{q}")
                for q in range(4)
            ]
            sbufs = [
                st_pool.tile([P, TC], bf16, tag=f"st{q}", name=f"st{q}")
                for q in range(3)
            ]

            warm = cst_pool.tile([P, P], bf16, tag="warm")
            nc.vector.memzero(warm[:])
            ps_w = pbufs[3]
            for wi in range(110):
                nc.tensor.matmul(ps_w[:, :64], warm[:], warm[:, :64],
                                 start=True, stop=True)

            # x + A stream on the sync queue in geometric k-chunks; mb0's W
            # tile prefetches on the scalar queue (idle until the stores).
            x_res = xres_pool.tile([P, KA, TC], bf16, tag="xres")
            a_res = cst_pool.tile([P, KA, 3, P], bf16, tag="a")
            w0_t = wbufs[0]
            nc.scalar.dma_start(out=w0_t[:], in_=w_d[0])
            # sync carries a0 + the x stream only; the remaining A chunks
            # ride the scalar queue (also HWDGE, idle until the stores) after
            # w0, so DMA-issue serialization never delays early x chunks
            kt0 = 0
            for ci, kch in enumerate((2, 2, 4, 4, 4, 8, 8)):
                sl = slice(kt0, kt0 + kch)
                aq = nc.sync if ci == 0 else nc.scalar
                aq.dma_start(out=a_res[:, sl], in_=a_d[:, sl])
                nc.sync.dma_start(out=x_res[:, sl], in_=x_d[:, sl])
                kt0 += kch
            gate_t = cst_pool.tile([P, TC], f32, tag="gate")
            nc.sync.dma_start(out=gate_t[:], in_=g_d[:])
            b_res = cst_pool.tile([P, MB, P], bf16, tag="b")
            nc.sync.dma_start(out=b_res[:], in_=b_d[:])

            # ------- Phase A (LoRA down-proj) with mb0's k-loop fused -------
            # 8 MMs per k-tile keeps PE consumption slower than the DMA
            # stream, so the PE never stalls at a chunk boundary.
            ps_a = pbufs[:3]
            ps_m0 = pbufs[3]
            for kk in range(KA):
                for i in range(3):
                    for j in range(NJ):
                        nc.tensor.matmul(
                            ps_a[i][:, j * 512:(j + 1) * 512],
                            a_res[:, kk, i, :],
                            x_res[:, kk, j * 512:(j + 1) * 512],
                            start=(kk == 0), stop=(kk == KA - 1),
                        )
                for j in range(NJ):
                    nc.tensor.matmul(
                        ps_m0[:, j * 512:(j + 1) * 512],
                        w0_t[:, kk, :],
                        x_res[:, kk, j * 512:(j + 1) * 512],
                        start=(kk == 0), stop=False,
                    )
            ag = []
            for i in range(3):
                ag_t = ag_pool.tile([P, TC], bf16, tag=f"ag{i}", name=f"ag{i}")
                nc.vector.tensor_mul(ag_t[:], ps_a[i][:], gate_t[:])
                ag.append(ag_t)

            # ------------- Main GEMM + fused LoRA up-proj -------------------
            for mb in range(MB):
                half = mb % 2
                if mb == 0:
                    pst = ps_m0
                else:
                    w_t = wbufs[mb % 4]
                    nc.sync.dma_start(out=w_t[:], in_=w_d[mb])
                    pst = pbufs[mb % 4]
                    for kk in range(KA):
                        for j in range(NJ):
                            nc.tensor.matmul(
                                pst[:, j * 512:(j + 1) * 512],
                                w_t[:, kk, :],
                                x_res[:, kk, j * 512:(j + 1) * 512],
                                start=(kk == 0), stop=False,
                            )
                agi = ag[I_OF_MB[mb]]
                for j in range(NJ):
                    nc.tensor.matmul(
                        pst[:, j * 512:(j + 1) * 512],
                        b_res[:, mb, :],
                        agi[:, j * 512:(j + 1) * 512],
                        start=False, stop=True,
                    )
                # single 2-bank cast + store per row-block; casts alternate
                # Vector/Scalar so consecutive row-blocks overlap
                st = sbufs[mb % 3]
                if half == 0:
                    nc.vector.tensor_copy(st[:], pst[:])
                else:
                    nc.scalar.copy(st[:], pst[:])
                nc.scalar.dma_start(out=y_d[mb], in_=st[:])

    nc.compile()
    return nc


def _get_nc():
    if "nc" not in _CACHE:
        _CACHE["nc"] = _build_nc()
    return _CACHE["nc"]


def _prep_in_maps(x, W, lora_A, lora_B_q, lora_B_k, lora_B_v, scaling, token_to_slot):
    f = np.float32
    # moving operand per token shard: [c][p(k), kt, t]
    x_sh = np.ascontiguousarray(
        np.asarray(x).astype(BF16).reshape(NCORES, TC, KA, P).transpose(0, 3, 2, 1))
    # W stationary (shared by all cores): [mb, p(k), kt, m]
    w_sh = np.ascontiguousarray(
        np.asarray(W).astype(BF16).reshape(MB, P, KA, P).transpose(0, 3, 2, 1))
    # LoRA A stationary: [p(k), kt, i, (s r)]
    a_sh = np.ascontiguousarray(
        np.asarray(lora_A).astype(BF16).reshape(S, 3, R, KA, P)
        .transpose(4, 3, 1, 0, 2).reshape(P, KA, 3, S * R))
    # LoRA B stationary (shared): [(s r), mb, m]
    bq = np.asarray(lora_B_q).astype(BF16)
    bk = np.asarray(lora_B_k).astype(BF16)
    bv = np.asarray(lora_B_v).astype(BF16)
    b_sh = np.ascontiguousarray(
        np.concatenate([bq, bk, bv], axis=1).transpose(0, 2, 1)
        .reshape(S * R, MB, P))
    # routing gate, expanded over ranks: [c][(s r), t]  (fp32, exact)
    slot = np.asarray(token_to_slot).reshape(NCORES, TC)
    g = (slot[:, None, :] == np.arange(S, dtype=slot.dtype)[None, :, None])
    g = g.astype(f) * np.asarray(scaling, dtype=f)[None, :, None]
    gate = np.ascontiguousarray(np.repeat(g, R, axis=1))

    in_maps = []
    for c in range(NCORES):
        in_maps.append({
            "x_sh": x_sh[c],
            "w_sh": w_sh,
            "a_sh": a_sh,
            "b_sh": b_sh,
            "gate": gate[c],
        })
    return in_maps


def _assemble(results):
    out = np.empty((T, D), dtype=np.float32)
    for c in range(NCORES):
        blk = np.asarray(results[c]["y_out"]).reshape(D, TC)
        out[c * TC:(c + 1) * TC, :] = blk.T
    return out


def _run(inputs, trace=False):
    from concourse.bass_utils import run_bass_kernel_spmd
    nc = _get_nc()
    in_maps = _prep_in_maps(**inputs)
    res = run_bass_kernel_spmd(
        nc, in_maps, core_ids=list(range(NCORES)), trace=trace)
    return res


def kernel(**inputs) -> np.ndarray:
    try:
        res = _run(inputs, trace=False)
    except Exception:
        # transient NRT/axon runtime hiccup (observed ~1/30 runs): retry once
        res = _run(inputs, trace=False)
    return _assemble(res.results)


if __name__ == "__main__":
    rng = np.random.default_rng(0)
    ins = {
        "x": rng.standard_normal((T, HID)).astype(np.float32),
        "W": (rng.standard_normal((D, HID)) * 0.02).astype(np.float32),
        "lora_A": (rng.standard_normal((S, 3, R, HID)) * 0.02).astype(np.float32),
        "lora_B_q": (rng.standard_normal((S, Q_SIZE, R)) * 0.02).astype(np.float32),
        "lora_B_k": (rng.standard_normal((S, KV_SIZE, R)) * 0.02).astype(np.float32),
        "lora_B_v": (rng.standard_normal((S, KV_SIZE, R)) * 0.02).astype(np.float32),
        "scaling": rng.uniform(0.5, 2.0, S).astype(np.float32),
        "token_to_slot": rng.integers(0, S, T).astype(np.int32),
    }
    out = kernel(**ins)
    print("out", out.shape, out.dtype)
